# revision 25
# baseline (speedup 1.0000x reference)
"""InterSliceAttention TRN2 kernel (v4).

Reference computation (per batch element b):
    curr = f_curr[b] as [N, C] tokens (N = H*W = 1024, C = 512)
    neigh = [f_prev[b]; f_next[b]] as [2N, C]
    Q = curr @ Wq.T ; K = neigh @ Wk.T ; V = neigh @ Wv.T
    8-head attention (hd = 64), softmax over 2N keys
    out = LayerNorm(curr + attn_out @ Wo.T) * gamma + beta   (LN over C)

Sharding: data-parallel over batch, one element per NeuronCore, weights
replicated. All activations stay channels-first ([C_part, token_free]).

v3 plan:
  - Host-side: weights pre-transposed; features/weights shipped as bf16
    (half the input DMA); DMAs spread over the two HWDGE queues (SP + ACT).
  - Q/K/V projections in bf16, evacuated to f32r (scores quality).
  - scores^T = K_h @ Q_h^T in f32r, [128 keys, 1024 q] per (head, key tile).
  - exp split: even units ACT table-exp -> fp8e5, odd units DVE Schraudolph
    int8 bitcast -> fp8e5. Both write slabs of a [128, 2, 1024] pair tile.
  - attnV: fp8 DoubleRow matmuls contract 256 keys per pass:
    [AO_h^T; rowsum] += [V_h | 1]^T(e4m3) @ e2(e5m2), accumulated over 8
    key-tile pairs into a single [65, 1024] PSUM accumulator per head.
  - PSUM: scores 3 x 2 banks + attn accumulator 1 x 2 banks = 8.
  - LN stats (sum x, sum x^2) share one [65, N] PSUM tile via interleaved
    accumulation groups (rows 0 and 64).
"""

import numpy as np

NUM_CORES = 8
B, C, H, W = 8, 512, 32, 32
N = H * W          # 1024 query tokens
N2 = 2 * N         # 2048 key tokens
HEADS = 8
HD = C // HEADS    # 64
SCALE = HD ** -0.5
LN_EPS = 1e-5
P = 128
CT = C // P        # 4 channel tiles
JT = N2 // P       # 16 key-token tiles
JP = JT // 2       # 8 key-tile pairs (DoubleRow contracts 256 keys)
FREE = 512
QC = N // FREE     # 2 query chunks
VP = 80            # padded per-(sub,head) lhsT stride in fp8 (step%16==0)

# Schraudolph fast-exp, fp8e5 (e5m2) flavor:
# exp(SCALE*x) ~= bitcast_e5m2(int8(A8*x + B8))
A_EXP8 = (1 << 2) / np.log(2.0) * SCALE
B_EXP8 = float((15 << 2) - 366393.0 / (1 << 21))

_CACHE = {}


def _emit(ctx, tc, io):
    import concourse.bass as bass
    from concourse import mybir

    nc = tc.nc
    f32 = mybir.dt.float32
    f32r = mybir.dt.float32r
    i8 = mybir.dt.int8
    bf16 = mybir.dt.bfloat16
    e4 = mybir.dt.float8e4
    e5 = mybir.dt.float8e5
    Alu = mybir.AluOpType
    Act = mybir.ActivationFunctionType
    DR = mybir.MatmulPerfMode.DoubleRow

    def F(ap):  # f32 view of an f32r tile for DVE/ACT consumers
        return ap.bitcast(f32)

    xc_d, xn_d, w_d, gamma_d, beta_d, y_d = io

    # ---------------- pools ----------------
    persist = ctx.enter_context(tc.tile_pool(name="persist", bufs=1))
    ps_a = ctx.enter_context(tc.tile_pool(name="ps_a", bufs=6, space="PSUM"))
    ps_b = ctx.enter_context(tc.tile_pool(name="ps_b", bufs=1, space="PSUM"))

    ones_col = persist.tile([P, 1], f32r, tag="ones")
    nc.vector.memset(F(ones_col[:]), 1.0)

    xc_sb = [persist.tile([P, N], bf16, tag=f"xc{i}", name=f"xc{i}") for i in range(CT)]
    qt_sb = [persist.tile([P, N], f32r, tag=f"qt{i}", name=f"qt{i}") for i in range(CT)]
    kt_sb = [persist.tile([P, N2], f32r, tag=f"kt{i}", name=f"kt{i}") for i in range(CT)]
    # V packed for DoubleRow: [keys mod 128, key-subtile, head, [V|1] padded]
    vv_sb = [persist.tile([P, 2, HEADS, VP], e4, tag=f"vv{jp}", name=f"vv{jp}")
             for jp in range(JP)]
    aot_sb = [persist.tile([P, N], bf16, tag=f"aot{i}", name=f"aot{i}") for i in range(CT)]
    wo_sb = [persist.tile([P, C], bf16, tag=f"wo{i}", name=f"wo{i}") for i in range(CT)]
    gamma_ct = [persist.tile([P, 1], f32, tag=f"g{i}", name=f"g{i}") for i in range(CT)]
    beta_ct = [persist.tile([P, 1], f32, tag=f"b{i}", name=f"b{i}") for i in range(CT)]
    eps_t = persist.tile([1, 1], f32, tag="eps")
    nc.vector.memset(eps_t[:], LN_EPS)

    # preload the exp activation table during the input DMA window
    dum = persist.tile([1, 8], f32, tag="dum")
    nc.vector.memset(dum[:], 1.0)
    nc.scalar.activation(dum[:], dum[:], Act.Exp)

    # ---------------- stage A: input DMA + QKV projections ----------------
    with tc.tile_pool(name="stageA", bufs=1) as a_pool:
        xn_sb = [a_pool.tile([P, N2], bf16, tag=f"xn{i}", name=f"xn{i}")
                 for i in range(CT)]
        wq_sb = [a_pool.tile([P, C], bf16, tag=f"wq{i}", name=f"wq{i}")
                 for i in range(CT)]
        wk_sb = [a_pool.tile([P, C], bf16, tag=f"wk{i}", name=f"wk{i}")
                 for i in range(CT)]
        wv_sb = [a_pool.tile([P, C], bf16, tag=f"wv{i}", name=f"wv{i}")
                 for i in range(CT)]

        # DMA spread over the two HWDGE queues.
        # sync: xn0, xn1, xc, wo ; scalar: wk, wv, wq, xn2, xn3, gamma, beta
        for i in range(2):
            nc.sync.dma_start(out=xn_sb[i][:], in_=xn_d[i * P:(i + 1) * P, :])
        for i in range(CT):
            nc.scalar.dma_start(out=wk_sb[i][:], in_=w_d["k"][i * P:(i + 1) * P, :])
        for i in range(CT):
            nc.scalar.dma_start(out=wv_sb[i][:], in_=w_d["v"][i * P:(i + 1) * P, :])
        for i in range(CT):
            nc.sync.dma_start(out=xc_sb[i][:], in_=xc_d[i * P:(i + 1) * P, :])
        for i in range(CT):
            nc.scalar.dma_start(out=wq_sb[i][:], in_=w_d["q"][i * P:(i + 1) * P, :])
        for i in range(2, CT):
            nc.scalar.dma_start(out=xn_sb[i][:], in_=xn_d[i * P:(i + 1) * P, :])
        for i in range(CT):
            nc.sync.dma_start(out=wo_sb[i][:], in_=w_d["o"][i * P:(i + 1) * P, :])
        for i in range(CT):
            nc.scalar.dma_start(out=gamma_ct[i][:], in_=gamma_d[i * P:(i + 1) * P, :])
            nc.scalar.dma_start(out=beta_ct[i][:], in_=beta_d[i * P:(i + 1) * P, :])

        # K projection: Kt[C,2N] = Wk @ Xn ; evacuation on ACT
        for mo in range(CT):
            for qc in range(N2 // FREE):
                ps = ps_a.tile([P, FREE], f32, tag="mm")
                for kt in range(CT):
                    nc.tensor.matmul(
                        ps[:],
                        wk_sb[kt][:, mo * P:(mo + 1) * P],
                        xn_sb[kt][:, qc * FREE:(qc + 1) * FREE],
                        start=(kt == 0), stop=(kt == CT - 1))
                nc.scalar.copy(out=kt_sb[mo][:, qc * FREE:(qc + 1) * FREE],
                               in_=ps[:])

        # V token-major, packed into vv[jp][:, sub, h, 0:64] (e4m3) + ones col;
        # evacuation on DVE
        for j in range(JT):
            ps = ps_a.tile([P, FREE], f32, tag="mm")
            for kt in range(CT):
                nc.tensor.matmul(
                    ps[:],
                    xn_sb[kt][:, j * P:(j + 1) * P],
                    wv_sb[kt][:],
                    start=(kt == 0), stop=(kt == CT - 1))
            jp, sub = divmod(j, 2)
            nc.vector.memset(vv_sb[jp][:, sub, :, HD], 1.0)
            nc.vector.tensor_copy(
                out=vv_sb[jp][:, sub, :, 0:HD],
                in_=ps[:].rearrange("p (h d) -> p h d", h=HEADS))

        # Q projection: Qt[C,N] = Wq @ Xc ; evacuation on ACT
        for mo in range(CT):
            for qc in range(QC):
                ps = ps_a.tile([P, FREE], f32, tag="mm")
                for kt in range(CT):
                    nc.tensor.matmul(
                        ps[:],
                        wq_sb[kt][:, mo * P:(mo + 1) * P],
                        xc_sb[kt][:, qc * FREE:(qc + 1) * FREE],
                        start=(kt == 0), stop=(kt == CT - 1))
                nc.scalar.copy(out=qt_sb[mo][:, qc * FREE:(qc + 1) * FREE],
                               in_=ps[:])

    # ---------------- stage C: attention ----------------
    # Stream of key-tile PAIRS p (head h = p // 8, pair jp = p % 8):
    #   scores(2p), scores(2p+1) -> exp into e2 slabs (ACT slab 0 / DVE slab 1)
    #   -> one fp8 DoubleRow attnV pass per pair (256 keys), emitted one pair
    #   late so the PE never waits on exp latency.
    with tc.tile_pool(name="expp", bufs=3) as exp_pool, \
         tc.tile_pool(name="normp", bufs=2) as norm_pool:
        PAIRS = HEADS * JP
        e2_of = [None] * PAIRS
        acc_of = {}

        def attn_pair(p):
            h, jp = divmod(p, JP)
            acc = acc_of[h]
            for qc in range(QC):
                nc.tensor.matmul(
                    acc[:, qc * FREE:(qc + 1) * FREE],
                    vv_sb[jp][:, :, h, 0:HD + 1],
                    e2_of[p][:, :, qc * FREE:(qc + 1) * FREE],
                    start=(jp == 0), stop=(jp == JP - 1),
                    perf_mode=DR)

        def normalize(h):
            hi, hr = h // 2, (h % 2) * HD
            acc = acc_of[h]
            recip = norm_pool.tile([1, N], f32, tag="r")
            nc.vector.reciprocal(recip[:], acc[HD:HD + 1, :])
            rb = norm_pool.tile([HD, N], f32, tag="rb")
            nc.gpsimd.partition_broadcast(rb[:], recip[:])
            if hr == 0:
                nc.vector.tensor_mul(aot_sb[hi][0:HD, :], acc[0:HD, :], rb[:])
            else:
                # odd heads land at rows 64:128 -> partition-shifting DMA
                ao = norm_pool.tile([HD, N], bf16, tag="ao")
                nc.vector.tensor_mul(ao[:], acc[0:HD, :], rb[:])
                nc.sync.dma_start(out=aot_sb[hi][hr:hr + HD, :], in_=ao[:])

        for p in range(PAIRS):
            h, jp = divmod(p, JP)
            hi, hr = h // 2, (h % 2) * HD
            if jp == 0:
                acc_of[h] = ps_b.tile([HD + 1, N], f32, tag="att", name=f"acc{h}")
            e2 = exp_pool.tile([P, 2, N], e5, tag="e2", name=f"e2_{p}")
            # scores + exp at [128, 512] chunk granularity: 6 one-bank score
            # buffers -> the exp->scores-reuse chain spans 1.5 pairs of PE work
            for sub in range(2):
                j = 2 * jp + sub
                for qc in range(QC):
                    s = ps_a.tile([P, FREE], f32, tag="mm", name=f"s{p}_{sub}{qc}")
                    nc.tensor.matmul(
                        s[:],
                        kt_sb[hi][hr:hr + HD, j * P:(j + 1) * P],
                        qt_sb[hi][hr:hr + HD, qc * FREE:(qc + 1) * FREE],
                        start=True, stop=True)
                    dst = e2[:, sub, qc * FREE:(qc + 1) * FREE]
                    if sub == 0:
                        nc.scalar.activation(dst, s[:], Act.Exp, scale=SCALE)
                    else:
                        nc.vector.tensor_scalar(
                            out=dst.bitcast(i8), in0=s[:], scalar1=A_EXP8,
                            scalar2=B_EXP8, op0=Alu.mult, op1=Alu.add)
            if p >= 1:
                attn_pair(p - 1)
                if (p - 1) % JP == JP - 1:
                    normalize((p - 1) // JP)
            e2_of[p] = e2
        attn_pair(PAIRS - 1)
        normalize(HEADS - 1)

    # ---------------- stage D: out_proj + residual + LayerNorm ----------------
    with tc.tile_pool(name="stageD", bufs=1) as d_pool, \
         tc.tile_pool(name="tmpD", bufs=2) as tmpd_pool:
        x_sb = [d_pool.tile([P, N], f32r, tag=f"x{i}", name=f"x{i}") for i in range(CT)]
        sq_sb = [d_pool.tile([P, N], f32r, tag=f"sq{i}", name=f"sq{i}") for i in range(CT)]

        for ct in range(CT):
            for qc in range(QC):
                ps_o = ps_a.tile([P, FREE], f32, tag="mm")
                for kt in range(CT):
                    nc.tensor.matmul(
                        ps_o[:],
                        wo_sb[kt][:, ct * P:(ct + 1) * P],
                        aot_sb[kt][:, qc * FREE:(qc + 1) * FREE],
                        start=(kt == 0), stop=(kt == CT - 1))
                # x = proj + residual
                sl = slice(qc * FREE, (qc + 1) * FREE)
                nc.vector.scalar_tensor_tensor(
                    out=x_sb[ct][:, sl], in0=ps_o[:], scalar=1.0,
                    in1=xc_sb[ct][:, sl], op0=Alu.mult, op1=Alu.add)
                nc.vector.tensor_mul(sq_sb[ct][:, sl], F(x_sb[ct][:, sl]),
                                     F(x_sb[ct][:, sl]))

        # LN stats: s1 (sum x) in the ps_b buffer, s2 (sum x^2) split into two
        # half-series in spare ps_a rotation slots; all at partition 0.
        s1 = ps_b.tile([1, N], f32, tag="att", name="s1")
        s2h = [ps_a.tile([1, FREE], f32, tag="mm", name=f"s2_{qc}")
               for qc in range(QC)]
        for ct in range(CT):
            for qc in range(QC):
                nc.tensor.matmul(
                    s1[:, qc * FREE:(qc + 1) * FREE],
                    ones_col[:], x_sb[ct][:, qc * FREE:(qc + 1) * FREE],
                    start=(ct == 0), stop=(ct == CT - 1))
                nc.tensor.matmul(
                    s2h[qc][:],
                    ones_col[:], sq_sb[ct][:, qc * FREE:(qc + 1) * FREE],
                    start=(ct == 0), stop=(ct == CT - 1))

        mu = d_pool.tile([1, N], f32, tag="mu")
        nc.vector.tensor_scalar_mul(mu[:], s1[:], 1.0 / C)
        mu2 = d_pool.tile([1, N], f32, tag="mu2")
        nc.vector.tensor_mul(mu2[:], mu[:], mu[:])
        var = d_pool.tile([1, N], f32, tag="var")
        for qc in range(QC):
            sl = slice(qc * FREE, (qc + 1) * FREE)
            nc.vector.scalar_tensor_tensor(
                out=var[:, sl], in0=s2h[qc][:], scalar=1.0 / C, in1=mu2[:, sl],
                op0=Alu.mult, op1=Alu.subtract)
        sd = d_pool.tile([1, N], f32, tag="sd")
        nc.scalar.activation(sd[:], var[:], Act.Sqrt, bias=eps_t[:])
        rinv = d_pool.tile([1, N], f32, tag="rinv")
        nc.vector.reciprocal(rinv[:], sd[:])
        mu_b = d_pool.tile([P, N], f32, tag="mub")
        nc.gpsimd.partition_broadcast(mu_b[:], mu[:])
        ri_b = d_pool.tile([P, N], f32, tag="rib")
        nc.gpsimd.partition_broadcast(ri_b[:], rinv[:])

        for ct in range(CT):
            t = tmpd_pool.tile([P, N], f32, tag="t")
            nc.gpsimd.tensor_sub(t[:], F(x_sb[ct][:]), mu_b[:])
            nc.gpsimd.tensor_mul(t[:], t[:], ri_b[:])
            y_sb = tmpd_pool.tile([P, N], f32, tag="y")
            nc.vector.tensor_scalar(
                out=y_sb[:], in0=t[:], scalar1=gamma_ct[ct][:],
                scalar2=beta_ct[ct][:], op0=Alu.mult, op1=Alu.add)
            nc.sync.dma_start(out=y_d[ct * P:(ct + 1) * P, :], in_=y_sb[:])


def _build(reps=1):
    from contextlib import ExitStack

    import concourse.tile as tile
    from concourse import bacc, mybir

    f32 = mybir.dt.float32
    bf16 = mybir.dt.bfloat16
    nc = bacc.Bacc("TRN2", target_bir_lowering=False, debug=False,
                   num_devices=NUM_CORES)
    xc_d = nc.dram_tensor("xc", [C, N], bf16, kind="ExternalInput").ap()
    xn_d = nc.dram_tensor("xn", [C, N2], bf16, kind="ExternalInput").ap()
    w_d = {k: nc.dram_tensor(f"w{k}t", [C, C], bf16, kind="ExternalInput").ap()
           for k in ("q", "k", "v", "o")}
    gamma_d = nc.dram_tensor("gamma", [C, 1], f32, kind="ExternalInput").ap()
    beta_d = nc.dram_tensor("beta", [C, 1], f32, kind="ExternalInput").ap()
    y_d = nc.dram_tensor("y", [C, N], f32, kind="ExternalOutput").ap()

    with tile.TileContext(nc) as tc:
        for _ in range(reps):
            with ExitStack() as ctx:
                _emit(ctx, tc, (xc_d, xn_d, w_d, gamma_d, beta_d, y_d))
    nc.compile()
    return nc


def _get_nc(reps=1):
    key = ("nc", reps)
    if key not in _CACHE:
        _CACHE[key] = _build(reps)
    return _CACHE[key]


def _bf16(a):
    import ml_dtypes
    return np.asarray(a, dtype=np.float32).astype(ml_dtypes.bfloat16)


def make_in_maps(f_curr, f_prev, f_next, Wq, Wk, Wv, Wo, gamma, beta):
    f_curr = np.asarray(f_curr, dtype=np.float32).reshape(B, C, N)
    f_prev = np.asarray(f_prev, dtype=np.float32).reshape(B, C, N)
    f_next = np.asarray(f_next, dtype=np.float32).reshape(B, C, N)
    xn = np.concatenate([f_prev, f_next], axis=2)  # [B, C, 2N]
    shared = {
        "wqt": _bf16(np.ascontiguousarray(np.asarray(Wq, dtype=np.float32).T)),
        "wkt": _bf16(np.ascontiguousarray(np.asarray(Wk, dtype=np.float32).T)),
        "wvt": _bf16(np.ascontiguousarray(np.asarray(Wv, dtype=np.float32).T)),
        "wot": _bf16(np.ascontiguousarray(np.asarray(Wo, dtype=np.float32).T)),
        "gamma": np.asarray(gamma, dtype=np.float32).reshape(C, 1),
        "beta": np.asarray(beta, dtype=np.float32).reshape(C, 1),
    }
    return [
        {"xc": _bf16(f_curr[b]), "xn": _bf16(xn[b]), **shared}
        for b in range(NUM_CORES)
    ]


def kernel(f_curr, f_prev, f_next, Wq, Wk, Wv, Wo, gamma, beta):
    from concourse.bass_utils import run_bass_kernel_spmd

    nc = _get_nc()
    in_maps = make_in_maps(f_curr, f_prev, f_next, Wq, Wk, Wv, Wo, gamma, beta)
    res = run_bass_kernel_spmd(nc, in_maps, list(range(NUM_CORES)))
    out = np.stack([res.results[b]["y"] for b in range(NUM_CORES)])
    return out.reshape(B, C, H, W).astype(np.float32)


# revision 30
# speedup vs baseline: 1.0441x; 1.0441x over previous
"""InterSliceAttention TRN2 kernel (v4).

Reference computation (per batch element b):
    curr = f_curr[b] as [N, C] tokens (N = H*W = 1024, C = 512)
    neigh = [f_prev[b]; f_next[b]] as [2N, C]
    Q = curr @ Wq.T ; K = neigh @ Wk.T ; V = neigh @ Wv.T
    8-head attention (hd = 64), softmax over 2N keys
    out = LayerNorm(curr + attn_out @ Wo.T) * gamma + beta   (LN over C)

Sharding: data-parallel over batch, one element per NeuronCore, weights
replicated. All activations stay channels-first ([C_part, token_free]).

v3 plan:
  - Host-side: weights pre-transposed; features/weights shipped as bf16
    (half the input DMA); DMAs spread over the two HWDGE queues (SP + ACT).
  - Q/K/V projections in bf16, evacuated to f32r (scores quality).
  - scores^T = K_h @ Q_h^T in f32r, [128 keys, 1024 q] per (head, key tile).
  - exp split: even units ACT table-exp -> fp8e5, odd units DVE Schraudolph
    int8 bitcast -> fp8e5. Both write slabs of a [128, 2, 1024] pair tile.
  - attnV: fp8 DoubleRow matmuls contract 256 keys per pass:
    [AO_h^T; rowsum] += [V_h | 1]^T(e4m3) @ e2(e5m2), accumulated over 8
    key-tile pairs into a single [65, 1024] PSUM accumulator per head.
  - PSUM: scores 3 x 2 banks + attn accumulator 1 x 2 banks = 8.
  - LN stats (sum x, sum x^2) share one [65, N] PSUM tile via interleaved
    accumulation groups (rows 0 and 64).
"""

import numpy as np

NUM_CORES = 8
B, C, H, W = 8, 512, 32, 32
N = H * W          # 1024 query tokens
N2 = 2 * N         # 2048 key tokens
HEADS = 8
HD = C // HEADS    # 64
SCALE = HD ** -0.5
LN_EPS = 1e-5
P = 128
CT = C // P        # 4 channel tiles
JT = N2 // P       # 16 key-token tiles
JP = JT // 2       # 8 key-tile pairs (DoubleRow contracts 256 keys)
FREE = 512
QC = N // FREE     # 2 query chunks
VP = 80            # padded per-(sub,head) lhsT stride in fp8 (step%16==0)

# Schraudolph fast-exp, fp8e5 (e5m2) flavor:
# exp(SCALE*x) ~= bitcast_e5m2(int8(A8*x + B8))
A_EXP8 = (1 << 2) / np.log(2.0) * SCALE
B_EXP8 = float((15 << 2) - 366393.0 / (1 << 21))

_CACHE = {}


def _emit(ctx, tc, io):
    import concourse.bass as bass
    from concourse import mybir

    nc = tc.nc
    f32 = mybir.dt.float32
    f32r = mybir.dt.float32r
    i8 = mybir.dt.int8
    bf16 = mybir.dt.bfloat16
    e4 = mybir.dt.float8e4
    e5 = mybir.dt.float8e5
    Alu = mybir.AluOpType
    Act = mybir.ActivationFunctionType
    DR = mybir.MatmulPerfMode.DoubleRow

    def F(ap):  # f32 view of an f32r tile for DVE/ACT consumers
        return ap.bitcast(f32)

    xc_d, xn_d, w_d, gamma_d, beta_d, y_d = io

    # ---------------- pools ----------------
    # bufs=2: consecutive kernel executions alternate persistent buffers, so
    # the next execution's input DMAs/projections overlap this one's tail
    persist = ctx.enter_context(tc.tile_pool(name="persist", bufs=2))
    ps_a = ctx.enter_context(tc.tile_pool(name="ps_a", bufs=3, space="PSUM"))
    ps_b = ctx.enter_context(tc.tile_pool(name="ps_b", bufs=1, space="PSUM"))

    ones_col = persist.tile([P, 1], bf16, tag="ones")
    nc.vector.memset(ones_col[:], 1.0)

    xc_sb = [persist.tile([P, N], bf16, tag=f"xc{i}", name=f"xc{i}") for i in range(CT)]
    qt_sb = [persist.tile([P, N], f32r, tag=f"qt{i}", name=f"qt{i}") for i in range(CT)]
    kt_sb = [persist.tile([P, N2], f32r, tag=f"kt{i}", name=f"kt{i}") for i in range(CT)]
    # V packed for DoubleRow: [keys mod 128, key-subtile, head, [V|1] padded]
    vv_sb = [persist.tile([P, 2, HEADS, VP], e4, tag=f"vv{jp}", name=f"vv{jp}")
             for jp in range(JP)]
    aot_sb = [persist.tile([P, N], bf16, tag=f"aot{i}", name=f"aot{i}") for i in range(CT)]
    wo_sb = [persist.tile([P, C], bf16, tag=f"wo{i}", name=f"wo{i}") for i in range(CT)]
    gamma_ct = [persist.tile([P, 1], f32, tag=f"g{i}", name=f"g{i}") for i in range(CT)]
    beta_ct = [persist.tile([P, 1], f32, tag=f"b{i}", name=f"b{i}") for i in range(CT)]
    eps_t = persist.tile([1, 1], f32, tag="eps")
    nc.vector.memset(eps_t[:], LN_EPS)

    # preload the exp activation table during the input DMA window
    dum = persist.tile([1, 8], f32, tag="dum")
    nc.vector.memset(dum[:], 1.0)
    nc.scalar.activation(dum[:], dum[:], Act.Exp)

    # ---------------- stage A: input DMA + QKV projections ----------------
    with tc.tile_pool(name="stageA", bufs=1) as a_pool:
        xn_sb = [a_pool.tile([P, N2], bf16, tag=f"xn{i}", name=f"xn{i}")
                 for i in range(CT)]
        wq_sb = [a_pool.tile([P, C], bf16, tag=f"wq{i}", name=f"wq{i}")
                 for i in range(CT)]
        wk_sb = [a_pool.tile([P, C], bf16, tag=f"wk{i}", name=f"wk{i}")
                 for i in range(CT)]
        wv_sb = [a_pool.tile([P, C], bf16, tag=f"wv{i}", name=f"wv{i}")
                 for i in range(CT)]

        # DMA spread over the two HWDGE queues.
        # sync: xn0, xn1, xc, wo ; scalar: wk, wv, wq, xn2, xn3, gamma, beta
        for i in range(2):
            nc.sync.dma_start(out=xn_sb[i][:], in_=xn_d[i * P:(i + 1) * P, :])
        for i in range(CT):
            nc.scalar.dma_start(out=wk_sb[i][:], in_=w_d["k"][i * P:(i + 1) * P, :])
        for i in range(CT):
            nc.scalar.dma_start(out=wv_sb[i][:], in_=w_d["v"][i * P:(i + 1) * P, :])
        for i in range(CT):
            nc.sync.dma_start(out=xc_sb[i][:], in_=xc_d[i * P:(i + 1) * P, :])
        for i in range(CT):
            nc.scalar.dma_start(out=wq_sb[i][:], in_=w_d["q"][i * P:(i + 1) * P, :])
        for i in range(2, CT):
            nc.scalar.dma_start(out=xn_sb[i][:], in_=xn_d[i * P:(i + 1) * P, :])
        for i in range(CT):
            nc.sync.dma_start(out=wo_sb[i][:], in_=w_d["o"][i * P:(i + 1) * P, :])
        for i in range(CT):
            nc.scalar.dma_start(out=gamma_ct[i][:], in_=gamma_d[i * P:(i + 1) * P, :])
            nc.scalar.dma_start(out=beta_ct[i][:], in_=beta_d[i * P:(i + 1) * P, :])

        # K projection: Kt[C,2N] = Wk @ Xn ; evacuation on ACT
        for mo in range(CT):
            for qc in range(N2 // FREE):
                ps = ps_a.tile([P, N], f32, tag="mm")
                for kt in range(CT):
                    nc.tensor.matmul(
                        ps[:, 0:FREE],
                        wk_sb[kt][:, mo * P:(mo + 1) * P],
                        xn_sb[kt][:, qc * FREE:(qc + 1) * FREE],
                        start=(kt == 0), stop=(kt == CT - 1))
                nc.scalar.copy(out=kt_sb[mo][:, qc * FREE:(qc + 1) * FREE],
                               in_=ps[:, 0:FREE])

        # V token-major, packed into vv[jp][:, sub, h, 0:64] (e4m3) + ones col;
        # evacuation on DVE
        for j in range(JT):
            ps = ps_a.tile([P, N], f32, tag="mm")
            for kt in range(CT):
                nc.tensor.matmul(
                    ps[:, 0:FREE],
                    xn_sb[kt][:, j * P:(j + 1) * P],
                    wv_sb[kt][:],
                    start=(kt == 0), stop=(kt == CT - 1))
            jp, sub = divmod(j, 2)
            nc.vector.memset(vv_sb[jp][:, sub, :, HD], 1.0)
            nc.vector.tensor_copy(
                out=vv_sb[jp][:, sub, :, 0:HD],
                in_=ps[:, 0:FREE].rearrange("p (h d) -> p h d", h=HEADS))

        # Q projection: Qt[C,N] = Wq @ Xc ; evacuation on ACT
        for mo in range(CT):
            for qc in range(QC):
                ps = ps_a.tile([P, N], f32, tag="mm")
                for kt in range(CT):
                    nc.tensor.matmul(
                        ps[:, 0:FREE],
                        wq_sb[kt][:, mo * P:(mo + 1) * P],
                        xc_sb[kt][:, qc * FREE:(qc + 1) * FREE],
                        start=(kt == 0), stop=(kt == CT - 1))
                nc.scalar.copy(out=qt_sb[mo][:, qc * FREE:(qc + 1) * FREE],
                               in_=ps[:, 0:FREE])

    # ---------------- stage C: attention ----------------
    # Stream of key-tile PAIRS p (head h = p // 8, pair jp = p % 8):
    #   scores(2p), scores(2p+1) -> exp into e2 slabs (ACT slab 0 / DVE slab 1)
    #   -> one fp8 DoubleRow attnV pass per pair (256 keys), emitted one pair
    #   late so the PE never waits on exp latency.
    with tc.tile_pool(name="expp", bufs=3) as exp_pool, \
         tc.tile_pool(name="normp", bufs=2) as norm_pool:
        PAIRS = HEADS * JP
        e2_of = [None] * PAIRS
        acc_of = {}

        def attn_pair(p):
            h, jp = divmod(p, JP)
            acc = acc_of[h]
            for qc in range(QC):
                nc.tensor.matmul(
                    acc[:, qc * FREE:(qc + 1) * FREE],
                    vv_sb[jp][:, :, h, 0:HD + 1],
                    e2_of[p][:, :, qc * FREE:(qc + 1) * FREE],
                    start=(jp == 0), stop=(jp == JP - 1),
                    perf_mode=DR)

        def normalize(h):
            hi, hr = h // 2, (h % 2) * HD
            acc = acc_of[h]
            recip = norm_pool.tile([1, N], f32, tag="r")
            nc.vector.reciprocal(recip[:], acc[HD:HD + 1, :])
            rb = norm_pool.tile([HD, N], f32, tag="rb")
            nc.gpsimd.partition_broadcast(rb[:], recip[:])
            if hr == 0:
                nc.vector.tensor_mul(aot_sb[hi][0:HD, :], acc[0:HD, :], rb[:])
            else:
                # odd heads land at rows 64:128 -> partition-shifting DMA
                ao = norm_pool.tile([HD, N], bf16, tag="ao")
                nc.vector.tensor_mul(ao[:], acc[0:HD, :], rb[:])
                nc.sync.dma_start(out=aot_sb[hi][hr:hr + HD, :], in_=ao[:])

        for p in range(PAIRS):
            h, jp = divmod(p, JP)
            hi, hr = h // 2, (h % 2) * HD
            if jp == 0:
                acc_of[h] = ps_b.tile([HD + 1, N], f32, tag="att", name=f"acc{h}")
            ss = []
            for sub in range(2):
                j = 2 * jp + sub
                s = ps_a.tile([P, N], f32, tag="mm", name=f"s{p}_{sub}")
                for qc in range(QC):
                    nc.tensor.matmul(
                        s[:, qc * FREE:(qc + 1) * FREE],
                        kt_sb[hi][hr:hr + HD, j * P:(j + 1) * P],
                        qt_sb[hi][hr:hr + HD, qc * FREE:(qc + 1) * FREE],
                        start=True, stop=True)
                ss.append(s)
            if p >= 1:
                attn_pair(p - 1)
                if (p - 1) % JP == JP - 1:
                    normalize((p - 1) // JP)
            e2 = exp_pool.tile([P, 2, N], e5, tag="e2", name=f"e2_{p}")
            nc.scalar.activation(e2[:, 0, :], ss[0][:], Act.Exp, scale=SCALE)
            nc.vector.tensor_scalar(
                out=e2[:, 1, :].bitcast(i8), in0=ss[1][:], scalar1=A_EXP8,
                scalar2=B_EXP8, op0=Alu.mult, op1=Alu.add)
            e2_of[p] = e2
        attn_pair(PAIRS - 1)
        normalize(HEADS - 1)

    # ---------------- stage D: out_proj + residual + LayerNorm ----------------
    with tc.tile_pool(name="stageD", bufs=1) as d_pool, \
         tc.tile_pool(name="tmpD", bufs=2) as tmpd_pool:
        x_sb = [d_pool.tile([P, N], bf16, tag=f"x{i}", name=f"x{i}") for i in range(CT)]
        sq_sb = [d_pool.tile([P, N], bf16, tag=f"sq{i}", name=f"sq{i}") for i in range(CT)]

        for ct in range(CT):
            ps_o = ps_a.tile([P, N], f32, tag="mm")
            for qc in range(QC):
                for kt in range(CT):
                    nc.tensor.matmul(
                        ps_o[:, qc * FREE:(qc + 1) * FREE],
                        wo_sb[kt][:, ct * P:(ct + 1) * P],
                        aot_sb[kt][:, qc * FREE:(qc + 1) * FREE],
                        start=(kt == 0), stop=(kt == CT - 1))
            # x = proj + residual
            nc.vector.scalar_tensor_tensor(
                out=x_sb[ct][:], in0=ps_o[:], scalar=1.0, in1=xc_sb[ct][:],
                op0=Alu.mult, op1=Alu.add)
            nc.vector.tensor_mul(sq_sb[ct][:], x_sb[ct][:], x_sb[ct][:])

        # LN stats: s1 (sum x) in the ps_b buffer, s2 (sum x^2) in a spare
        # ps_a rotation slot; both [1, N] at partition 0 of their own banks.
        s1 = ps_b.tile([1, N], f32, tag="att", name="s1")
        s2 = ps_a.tile([1, N], f32, tag="mm", name="s2")
        for ct in range(CT):
            for qc in range(QC):
                nc.tensor.matmul(
                    s1[:, qc * FREE:(qc + 1) * FREE],
                    ones_col[:], x_sb[ct][:, qc * FREE:(qc + 1) * FREE],
                    start=(ct == 0), stop=(ct == CT - 1))
                nc.tensor.matmul(
                    s2[:, qc * FREE:(qc + 1) * FREE],
                    ones_col[:], sq_sb[ct][:, qc * FREE:(qc + 1) * FREE],
                    start=(ct == 0), stop=(ct == CT - 1))

        lp = nc.allow_low_precision(reason="LN stats bf16, 0.4%")
        lp.__enter__()
        mu = d_pool.tile([1, N], bf16, tag="mu")
        nc.vector.tensor_scalar_mul(mu[:], s1[:], 1.0 / C)
        mu2 = d_pool.tile([1, N], bf16, tag="mu2")
        nc.vector.tensor_mul(mu2[:], mu[:], mu[:])
        var = d_pool.tile([1, N], f32, tag="var")
        nc.vector.scalar_tensor_tensor(
            out=var[:], in0=s2[:], scalar=1.0 / C, in1=mu2[:],
            op0=Alu.mult, op1=Alu.subtract)
        sd = d_pool.tile([1, N], f32, tag="sd")
        nc.scalar.activation(sd[:], var[:], Act.Sqrt, bias=eps_t[:])
        rinv = d_pool.tile([1, N], bf16, tag="rinv")
        nc.vector.reciprocal(rinv[:], sd[:])
        mu_b = d_pool.tile([P, N], bf16, tag="mub")
        nc.gpsimd.partition_broadcast(mu_b[:], mu[:])
        ri_b = d_pool.tile([P, N], bf16, tag="rib")
        nc.gpsimd.partition_broadcast(ri_b[:], rinv[:])
        lp.__exit__(None, None, None)

        for ct in range(CT):
            t = tmpd_pool.tile([P, N], bf16, tag="t")
            nc.gpsimd.tensor_sub(t[:], x_sb[ct][:], mu_b[:])
            nc.gpsimd.tensor_mul(t[:], t[:], ri_b[:])
            y_sb = tmpd_pool.tile([P, N], f32, tag="y")
            nc.vector.tensor_scalar(
                out=y_sb[:], in0=t[:], scalar1=gamma_ct[ct][:],
                scalar2=beta_ct[ct][:], op0=Alu.mult, op1=Alu.add)
            nc.sync.dma_start(out=y_d[ct * P:(ct + 1) * P, :], in_=y_sb[:])


def _build(reps=1):
    from contextlib import ExitStack

    import concourse.tile as tile
    from concourse import bacc, mybir

    f32 = mybir.dt.float32
    bf16 = mybir.dt.bfloat16
    nc = bacc.Bacc("TRN2", target_bir_lowering=False, debug=False,
                   num_devices=NUM_CORES)
    xc_d = nc.dram_tensor("xc", [C, N], bf16, kind="ExternalInput").ap()
    xn_d = nc.dram_tensor("xn", [C, N2], bf16, kind="ExternalInput").ap()
    w_d = {k: nc.dram_tensor(f"w{k}t", [C, C], bf16, kind="ExternalInput").ap()
           for k in ("q", "k", "v", "o")}
    gamma_d = nc.dram_tensor("gamma", [C, 1], f32, kind="ExternalInput").ap()
    beta_d = nc.dram_tensor("beta", [C, 1], f32, kind="ExternalInput").ap()
    y_d = nc.dram_tensor("y", [C, N], f32, kind="ExternalOutput").ap()

    with tile.TileContext(nc) as tc:
        for _ in range(reps):
            with ExitStack() as ctx:
                _emit(ctx, tc, (xc_d, xn_d, w_d, gamma_d, beta_d, y_d))
    nc.compile()
    return nc


def _get_nc(reps=1):
    key = ("nc", reps)
    if key not in _CACHE:
        _CACHE[key] = _build(reps)
    return _CACHE[key]


def _bf16(a):
    import ml_dtypes
    return np.asarray(a, dtype=np.float32).astype(ml_dtypes.bfloat16)


def make_in_maps(f_curr, f_prev, f_next, Wq, Wk, Wv, Wo, gamma, beta):
    f_curr = np.asarray(f_curr, dtype=np.float32).reshape(B, C, N)
    f_prev = np.asarray(f_prev, dtype=np.float32).reshape(B, C, N)
    f_next = np.asarray(f_next, dtype=np.float32).reshape(B, C, N)
    xn = np.concatenate([f_prev, f_next], axis=2)  # [B, C, 2N]
    shared = {
        "wqt": _bf16(np.ascontiguousarray(np.asarray(Wq, dtype=np.float32).T)),
        "wkt": _bf16(np.ascontiguousarray(np.asarray(Wk, dtype=np.float32).T)),
        "wvt": _bf16(np.ascontiguousarray(np.asarray(Wv, dtype=np.float32).T)),
        "wot": _bf16(np.ascontiguousarray(np.asarray(Wo, dtype=np.float32).T)),
        "gamma": np.asarray(gamma, dtype=np.float32).reshape(C, 1),
        "beta": np.asarray(beta, dtype=np.float32).reshape(C, 1),
    }
    return [
        {"xc": _bf16(f_curr[b]), "xn": _bf16(xn[b]), **shared}
        for b in range(NUM_CORES)
    ]


def kernel(f_curr, f_prev, f_next, Wq, Wk, Wv, Wo, gamma, beta):
    from concourse.bass_utils import run_bass_kernel_spmd

    nc = _get_nc()
    in_maps = make_in_maps(f_curr, f_prev, f_next, Wq, Wk, Wv, Wo, gamma, beta)
    res = run_bass_kernel_spmd(nc, in_maps, list(range(NUM_CORES)))
    out = np.stack([res.results[b]["y"] for b in range(NUM_CORES)])
    return out.reshape(B, C, H, W).astype(np.float32)


# revision 33
# speedup vs baseline: 1.2333x; 1.1812x over previous
"""InterSliceAttention TRN2 kernel (v4).

Reference computation (per batch element b):
    curr = f_curr[b] as [N, C] tokens (N = H*W = 1024, C = 512)
    neigh = [f_prev[b]; f_next[b]] as [2N, C]
    Q = curr @ Wq.T ; K = neigh @ Wk.T ; V = neigh @ Wv.T
    8-head attention (hd = 64), softmax over 2N keys
    out = LayerNorm(curr + attn_out @ Wo.T) * gamma + beta   (LN over C)

Sharding: data-parallel over batch, one element per NeuronCore, weights
replicated. All activations stay channels-first ([C_part, token_free]).

v3 plan:
  - Host-side: weights pre-transposed; features/weights shipped as bf16
    (half the input DMA); DMAs spread over the two HWDGE queues (SP + ACT).
  - Q/K/V projections in bf16, evacuated to f32r (scores quality).
  - scores^T = K_h @ Q_h^T in f32r, [128 keys, 1024 q] per (head, key tile).
  - exp split: even units ACT table-exp -> fp8e5, odd units DVE Schraudolph
    int8 bitcast -> fp8e5. Both write slabs of a [128, 2, 1024] pair tile.
  - attnV: fp8 DoubleRow matmuls contract 256 keys per pass:
    [AO_h^T; rowsum] += [V_h | 1]^T(e4m3) @ e2(e5m2), accumulated over 8
    key-tile pairs into a single [65, 1024] PSUM accumulator per head.
  - PSUM: scores 3 x 2 banks + attn accumulator 1 x 2 banks = 8.
  - LN stats (sum x, sum x^2) share one [65, N] PSUM tile via interleaved
    accumulation groups (rows 0 and 64).
"""

import numpy as np

NUM_CORES = 8
B, C, H, W = 8, 512, 32, 32
N = H * W          # 1024 query tokens
N2 = 2 * N         # 2048 key tokens
HEADS = 8
HD = C // HEADS    # 64
SCALE = HD ** -0.5
LN_EPS = 1e-5
P = 128
CT = C // P        # 4 channel tiles
JT = N2 // P       # 16 key-token tiles
JP = JT // 2       # 8 key-tile pairs (DoubleRow contracts 256 keys)
FREE = 512
QC = N // FREE     # 2 query chunks
VP = 80            # padded per-(sub,head) lhsT stride in fp8 (step%16==0)

# Schraudolph fast-exp, fp8e5 (e5m2) flavor:
# exp(SCALE*x) ~= bitcast_e5m2(int8(A8*x + B8))
A_EXP8 = (1 << 2) / np.log(2.0) * SCALE
B_EXP8 = float((15 << 2) - 366393.0 / (1 << 21))

_CACHE = {}


def _emit(ctx, tc, io):
    import concourse.bass as bass
    from concourse import mybir

    nc = tc.nc
    f32 = mybir.dt.float32
    f32r = mybir.dt.float32r
    i8 = mybir.dt.int8
    bf16 = mybir.dt.bfloat16
    e4 = mybir.dt.float8e4
    e5 = mybir.dt.float8e5
    Alu = mybir.AluOpType
    Act = mybir.ActivationFunctionType
    DR = mybir.MatmulPerfMode.DoubleRow

    def F(ap):  # f32 view of an f32r tile for DVE/ACT consumers
        return ap.bitcast(f32)

    xc_d, xn_d, w_d, gamma_d, beta_d, y_d = io

    # ---------------- pools ----------------
    persist = ctx.enter_context(tc.tile_pool(name="persist", bufs=1))
    ps_a = ctx.enter_context(tc.tile_pool(name="ps_a", bufs=3, space="PSUM"))
    ps_b = ctx.enter_context(tc.tile_pool(name="ps_b", bufs=1, space="PSUM"))

    ones_col = persist.tile([P, 1], f32r, tag="ones")
    nc.vector.memset(F(ones_col[:]), 1.0)

    xc_sb = [persist.tile([P, N], bf16, tag=f"xc{i}", name=f"xc{i}") for i in range(CT)]
    qt_sb = [persist.tile([P, N], f32r, tag=f"qt{i}", name=f"qt{i}") for i in range(CT)]
    kt_sb = [persist.tile([P, N2], f32r, tag=f"kt{i}", name=f"kt{i}") for i in range(CT)]
    # V packed for DoubleRow: [keys mod 128, key-subtile, head, [V|1] padded]
    vv_sb = [persist.tile([P, 2, HEADS, VP], e4, tag=f"vv{jp}", name=f"vv{jp}")
             for jp in range(JP)]
    aot_sb = [persist.tile([P, N], bf16, tag=f"aot{i}", name=f"aot{i}") for i in range(CT)]
    wo_sb = [persist.tile([P, C], bf16, tag=f"wo{i}", name=f"wo{i}") for i in range(CT)]
    gamma_ct = [persist.tile([P, 1], f32, tag=f"g{i}", name=f"g{i}") for i in range(CT)]
    beta_ct = [persist.tile([P, 1], f32, tag=f"b{i}", name=f"b{i}") for i in range(CT)]
    eps_t = persist.tile([1, 1], f32, tag="eps")
    nc.vector.memset(eps_t[:], LN_EPS)

    # preload the exp activation table during the input DMA window
    dum = persist.tile([1, 8], f32, tag="dum")
    nc.vector.memset(dum[:], 1.0)
    nc.scalar.activation(dum[:], dum[:], Act.Exp)

    # ---------------- stage A: input DMA + QKV projections ----------------
    with tc.tile_pool(name="stageA", bufs=1) as a_pool:
        xn_sb = [a_pool.tile([P, N2], bf16, tag=f"xn{i}", name=f"xn{i}")
                 for i in range(CT)]
        wq_sb = [a_pool.tile([P, C], bf16, tag=f"wq{i}", name=f"wq{i}")
                 for i in range(CT)]
        wk_sb = [a_pool.tile([P, C], bf16, tag=f"wk{i}", name=f"wk{i}")
                 for i in range(CT)]
        wv_sb = [a_pool.tile([P, C], bf16, tag=f"wv{i}", name=f"wv{i}")
                 for i in range(CT)]

        # DMA spread over the two HWDGE queues.
        # sync: xn0, xn1, xc, wo ; scalar: wk, wv, wq, xn2, xn3, gamma, beta
        for i in range(2):
            nc.sync.dma_start(out=xn_sb[i][:], in_=xn_d[i * P:(i + 1) * P, :])
        for i in range(CT):
            nc.scalar.dma_start(out=wk_sb[i][:], in_=w_d["k"][i * P:(i + 1) * P, :])
        for i in range(CT):
            nc.scalar.dma_start(out=wv_sb[i][:], in_=w_d["v"][i * P:(i + 1) * P, :])
        for i in range(CT):
            nc.sync.dma_start(out=xc_sb[i][:], in_=xc_d[i * P:(i + 1) * P, :])
        for i in range(CT):
            nc.scalar.dma_start(out=wq_sb[i][:], in_=w_d["q"][i * P:(i + 1) * P, :])
        for i in range(2, CT):
            nc.scalar.dma_start(out=xn_sb[i][:], in_=xn_d[i * P:(i + 1) * P, :])
        for i in range(CT):
            nc.sync.dma_start(out=wo_sb[i][:], in_=w_d["o"][i * P:(i + 1) * P, :])
        for i in range(CT):
            nc.scalar.dma_start(out=gamma_ct[i][:], in_=gamma_d[i * P:(i + 1) * P, :])
            nc.scalar.dma_start(out=beta_ct[i][:], in_=beta_d[i * P:(i + 1) * P, :])

        # K projection: Kt[C,2N] = Wk @ Xn ; evacuation on ACT
        for mo in range(CT):
            for qc in range(N2 // FREE):
                ps = ps_a.tile([P, N], f32, tag="mm")
                for kt in range(CT):
                    nc.tensor.matmul(
                        ps[:, 0:FREE],
                        wk_sb[kt][:, mo * P:(mo + 1) * P],
                        xn_sb[kt][:, qc * FREE:(qc + 1) * FREE],
                        start=(kt == 0), stop=(kt == CT - 1))
                nc.scalar.copy(out=kt_sb[mo][:, qc * FREE:(qc + 1) * FREE],
                               in_=ps[:, 0:FREE])

        # V token-major, packed into vv[jp][:, sub, h, 0:64] (e4m3) + ones col;
        # evacuation on DVE
        for j in range(JT):
            ps = ps_a.tile([P, N], f32, tag="mm")
            for kt in range(CT):
                nc.tensor.matmul(
                    ps[:, 0:FREE],
                    xn_sb[kt][:, j * P:(j + 1) * P],
                    wv_sb[kt][:],
                    start=(kt == 0), stop=(kt == CT - 1))
            jp, sub = divmod(j, 2)
            nc.vector.memset(vv_sb[jp][:, sub, :, HD], 1.0)
            nc.vector.tensor_copy(
                out=vv_sb[jp][:, sub, :, 0:HD],
                in_=ps[:, 0:FREE].rearrange("p (h d) -> p h d", h=HEADS))

        # Q projection: Qt[C,N] = Wq @ Xc ; evacuation on ACT
        for mo in range(CT):
            for qc in range(QC):
                ps = ps_a.tile([P, N], f32, tag="mm")
                for kt in range(CT):
                    nc.tensor.matmul(
                        ps[:, 0:FREE],
                        wq_sb[kt][:, mo * P:(mo + 1) * P],
                        xc_sb[kt][:, qc * FREE:(qc + 1) * FREE],
                        start=(kt == 0), stop=(kt == CT - 1))
                nc.scalar.copy(out=qt_sb[mo][:, qc * FREE:(qc + 1) * FREE],
                               in_=ps[:, 0:FREE])

    # ---------------- stage C: attention ----------------
    # Stream of key-tile PAIRS p (head h = p // 8, pair jp = p % 8):
    #   scores(2p), scores(2p+1) -> exp into e2 slabs (ACT slab 0 / DVE slab 1)
    #   -> one fp8 DoubleRow attnV pass per pair (256 keys), emitted one pair
    #   late so the PE never waits on exp latency.
    with tc.tile_pool(name="expp", bufs=3) as exp_pool, \
         tc.tile_pool(name="normp", bufs=2) as norm_pool:
        PAIRS = HEADS * JP
        e2_of = [None] * PAIRS
        acc_of = {}

        def attn_pair(p):
            h, jp = divmod(p, JP)
            acc = acc_of[h]
            for qc in range(QC):
                nc.tensor.matmul(
                    acc[:, qc * FREE:(qc + 1) * FREE],
                    vv_sb[jp][:, :, h, 0:HD + 1],
                    e2_of[p][:, :, qc * FREE:(qc + 1) * FREE],
                    start=(jp == 0), stop=(jp == JP - 1),
                    perf_mode=DR)

        def normalize(h):
            hi, hr = h // 2, (h % 2) * HD
            acc = acc_of[h]
            recip = norm_pool.tile([1, N], f32, tag="r")
            nc.vector.reciprocal(recip[:], acc[HD:HD + 1, :])
            rb = norm_pool.tile([HD, N], f32, tag="rb")
            nc.gpsimd.partition_broadcast(rb[:], recip[:])
            if hr == 0:
                nc.vector.tensor_mul(aot_sb[hi][0:HD, :], acc[0:HD, :], rb[:])
            else:
                # odd heads land at rows 64:128 -> partition-shifting DMA
                ao = norm_pool.tile([HD, N], bf16, tag="ao")
                nc.vector.tensor_mul(ao[:], acc[0:HD, :], rb[:])
                nc.sync.dma_start(out=aot_sb[hi][hr:hr + HD, :], in_=ao[:])

        for p in range(PAIRS):
            h, jp = divmod(p, JP)
            hi, hr = h // 2, (h % 2) * HD
            if jp == 0:
                acc_of[h] = ps_b.tile([HD + 1, N], f32, tag="att", name=f"acc{h}")
            ss = []
            for sub in range(2):
                j = 2 * jp + sub
                s = ps_a.tile([P, N], f32, tag="mm", name=f"s{p}_{sub}")
                for qc in range(QC):
                    nc.tensor.matmul(
                        s[:, qc * FREE:(qc + 1) * FREE],
                        kt_sb[hi][hr:hr + HD, j * P:(j + 1) * P],
                        qt_sb[hi][hr:hr + HD, qc * FREE:(qc + 1) * FREE],
                        start=True, stop=True)
                ss.append(s)
            if p >= 1:
                attn_pair(p - 1)
                if (p - 1) % JP == JP - 1:
                    normalize((p - 1) // JP)
            e2 = exp_pool.tile([P, 2, N], e5, tag="e2", name=f"e2_{p}")
            nc.scalar.activation(e2[:, 0, :], ss[0][:], Act.Exp, scale=SCALE)
            nc.vector.tensor_scalar(
                out=e2[:, 1, :].bitcast(i8), in0=ss[1][:], scalar1=A_EXP8,
                scalar2=B_EXP8, op0=Alu.mult, op1=Alu.add)
            e2_of[p] = e2
        attn_pair(PAIRS - 1)
        normalize(HEADS - 1)

    # ---------------- stage D: out_proj + residual + LayerNorm ----------------
    with tc.tile_pool(name="stageD", bufs=1) as d_pool, \
         tc.tile_pool(name="tmpD", bufs=2) as tmpd_pool:
        x_sb = [d_pool.tile([P, N], f32r, tag=f"x{i}", name=f"x{i}") for i in range(CT)]
        sq_sb = [d_pool.tile([P, N], f32r, tag=f"sq{i}", name=f"sq{i}") for i in range(CT)]

        for ct in range(CT):
            ps_o = ps_a.tile([P, N], f32, tag="mm")
            for qc in range(QC):
                for kt in range(CT):
                    nc.tensor.matmul(
                        ps_o[:, qc * FREE:(qc + 1) * FREE],
                        wo_sb[kt][:, ct * P:(ct + 1) * P],
                        aot_sb[kt][:, qc * FREE:(qc + 1) * FREE],
                        start=(kt == 0), stop=(kt == CT - 1))
            # x = proj + residual
            nc.vector.scalar_tensor_tensor(
                out=x_sb[ct][:], in0=ps_o[:], scalar=1.0, in1=xc_sb[ct][:],
                op0=Alu.mult, op1=Alu.add)
            nc.vector.tensor_mul(sq_sb[ct][:], F(x_sb[ct][:]), F(x_sb[ct][:]))

        # LN stats: s1 (sum x) in the ps_b buffer, s2 (sum x^2) in a spare
        # ps_a rotation slot; both [1, N] at partition 0 of their own banks.
        s1 = ps_b.tile([1, N], f32, tag="att", name="s1")
        s2 = ps_a.tile([1, N], f32, tag="mm", name="s2")
        for ct in range(CT):
            for qc in range(QC):
                nc.tensor.matmul(
                    s1[:, qc * FREE:(qc + 1) * FREE],
                    ones_col[:], x_sb[ct][:, qc * FREE:(qc + 1) * FREE],
                    start=(ct == 0), stop=(ct == CT - 1))
                nc.tensor.matmul(
                    s2[:, qc * FREE:(qc + 1) * FREE],
                    ones_col[:], sq_sb[ct][:, qc * FREE:(qc + 1) * FREE],
                    start=(ct == 0), stop=(ct == CT - 1))

        mu = d_pool.tile([1, N], f32, tag="mu")
        nc.vector.tensor_scalar_mul(mu[:], s1[:], 1.0 / C)
        mu2 = d_pool.tile([1, N], f32, tag="mu2")
        nc.vector.tensor_mul(mu2[:], mu[:], mu[:])
        var = d_pool.tile([1, N], f32, tag="var")
        nc.vector.scalar_tensor_tensor(
            out=var[:], in0=s2[:], scalar=1.0 / C, in1=mu2[:],
            op0=Alu.mult, op1=Alu.subtract)
        sd = d_pool.tile([1, N], f32, tag="sd")
        nc.scalar.activation(sd[:], var[:], Act.Sqrt, bias=eps_t[:])
        rinv = d_pool.tile([1, N], f32, tag="rinv")
        nc.vector.reciprocal(rinv[:], sd[:])
        mu_b = d_pool.tile([P, N], f32, tag="mub")
        nc.gpsimd.partition_broadcast(mu_b[:], mu[:])
        ri_b = d_pool.tile([P, N], f32, tag="rib")
        nc.gpsimd.partition_broadcast(ri_b[:], rinv[:])

        for ct in range(CT):
            t = tmpd_pool.tile([P, N], f32, tag="t")
            nc.gpsimd.tensor_sub(t[:], F(x_sb[ct][:]), mu_b[:])
            nc.gpsimd.tensor_mul(t[:], t[:], ri_b[:])
            y_sb = tmpd_pool.tile([P, N], f32, tag="y")
            nc.vector.tensor_scalar(
                out=y_sb[:], in0=t[:], scalar1=gamma_ct[ct][:],
                scalar2=beta_ct[ct][:], op0=Alu.mult, op1=Alu.add)
            nc.sync.dma_start(out=y_d[ct * P:(ct + 1) * P, :], in_=y_sb[:])


def _build(reps=1):
    from contextlib import ExitStack

    import concourse.tile as tile
    from concourse import bacc, mybir

    f32 = mybir.dt.float32
    bf16 = mybir.dt.bfloat16
    nc = bacc.Bacc("TRN2", target_bir_lowering=False, debug=False,
                   num_devices=NUM_CORES)
    xc_d = nc.dram_tensor("xc", [C, N], bf16, kind="ExternalInput").ap()
    xn_d = nc.dram_tensor("xn", [C, N2], bf16, kind="ExternalInput").ap()
    w_d = {k: nc.dram_tensor(f"w{k}t", [C, C], bf16, kind="ExternalInput").ap()
           for k in ("q", "k", "v", "o")}
    gamma_d = nc.dram_tensor("gamma", [C, 1], f32, kind="ExternalInput").ap()
    beta_d = nc.dram_tensor("beta", [C, 1], f32, kind="ExternalInput").ap()
    y_d = nc.dram_tensor("y", [C, N], f32, kind="ExternalOutput").ap()

    with tile.TileContext(nc) as tc:
        for _ in range(reps):
            with ExitStack() as ctx:
                _emit(ctx, tc, (xc_d, xn_d, w_d, gamma_d, beta_d, y_d))
    nc.compile()
    return nc


def _get_nc(reps=1):
    key = ("nc", reps)
    if key not in _CACHE:
        _CACHE[key] = _build(reps)
    return _CACHE[key]


def _bf16(a):
    import ml_dtypes
    return np.asarray(a, dtype=np.float32).astype(ml_dtypes.bfloat16)


def make_in_maps(f_curr, f_prev, f_next, Wq, Wk, Wv, Wo, gamma, beta):
    f_curr = np.asarray(f_curr, dtype=np.float32).reshape(B, C, N)
    f_prev = np.asarray(f_prev, dtype=np.float32).reshape(B, C, N)
    f_next = np.asarray(f_next, dtype=np.float32).reshape(B, C, N)
    xn = np.concatenate([f_prev, f_next], axis=2)  # [B, C, 2N]
    shared = {
        "wqt": _bf16(np.ascontiguousarray(np.asarray(Wq, dtype=np.float32).T)),
        "wkt": _bf16(np.ascontiguousarray(np.asarray(Wk, dtype=np.float32).T)),
        "wvt": _bf16(np.ascontiguousarray(np.asarray(Wv, dtype=np.float32).T)),
        "wot": _bf16(np.ascontiguousarray(np.asarray(Wo, dtype=np.float32).T)),
        "gamma": np.asarray(gamma, dtype=np.float32).reshape(C, 1),
        "beta": np.asarray(beta, dtype=np.float32).reshape(C, 1),
    }
    return [
        {"xc": _bf16(f_curr[b]), "xn": _bf16(xn[b]), **shared}
        for b in range(NUM_CORES)
    ]


def kernel(f_curr, f_prev, f_next, Wq, Wk, Wv, Wo, gamma, beta):
    from concourse.bass_utils import run_bass_kernel_spmd

    nc = _get_nc()
    in_maps = make_in_maps(f_curr, f_prev, f_next, Wq, Wk, Wv, Wo, gamma, beta)
    res = run_bass_kernel_spmd(nc, in_maps, list(range(NUM_CORES)))
    out = np.stack([res.results[b]["y"] for b in range(NUM_CORES)])
    return out.reshape(B, C, H, W).astype(np.float32)
